# revision 1
# baseline (speedup 1.0000x reference)
"""AttentionWithBinding distributed Bass kernel for 8 TRN2 NeuronCores.

Sharding: 8 cores = 2 batches x 4 head-groups (4 heads / 256 dims each).
Per core: q/k/v projections (weight- or x-stationary matmuls from a
host-pre-transposed xT), flash-style attention in scoresT [sk, sq]
orientation, softmax exp on ScalarE with the additive binding bias folded
in as a host-precomputed exp(0.5*binding.T) bf16 multiplier on VectorE,
row-sums fused into the attn@v matmul via a ones-column on v, and the
per-head o-projection partials. Host sums the 4 partials per batch and
adds the analytic bias vector bv@Wo + bo (softmax rows sum to 1, so the
v-bias passes through attention unchanged).
"""

import sys

sys.path.insert(0, "/opt/trn_rl_repo")

import numpy as np
import ml_dtypes
from contextlib import ExitStack

BF16 = ml_dtypes.bfloat16

B, S, D = 2, 2048, 1024
H, HD = 16, 64
HPC = 4  # heads per core
DHC = HPC * HD  # 256 head dims per core
SCALE = HD ** -0.5
NCORES = 8
KT = D // 128  # 8 contraction tiles over D
ST = S // 128  # 16 tiles over S
CH = 512  # free-dim chunk (one PSUM bank of f32)
NQ = S // CH  # 4 query chunks

_graph_cache = {}


def _build():
    import concourse.bacc as bacc
    import concourse.mybir as mybir
    from concourse import tile

    f32 = mybir.dt.float32
    bf16 = mybir.dt.bfloat16
    f8 = mybir.dt.float8e5
    AF = mybir.ActivationFunctionType

    nc = bacc.Bacc(None)

    xT_e = nc.declare_dram_parameter("xT", [D, S], bf16, isOutput=False)
    wq_e = nc.declare_dram_parameter("wq", [D, DHC], bf16, isOutput=False)
    wk_e = nc.declare_dram_parameter("wk", [D, DHC], bf16, isOutput=False)
    wv_e = nc.declare_dram_parameter("wv", [D, DHC], bf16, isOutput=False)
    wo_e = nc.declare_dram_parameter("wo", [DHC, D], bf16, isOutput=False)
    eb_e = nc.declare_dram_parameter("expbT", [S, S], bf16, isOutput=False)
    bq_e = nc.declare_dram_parameter("bq", [DHC, 1], f32, isOutput=False)
    bk_e = nc.declare_dram_parameter("bk", [DHC, 1], f32, isOutput=False)
    out_e = nc.declare_dram_parameter("out", [S, D], bf16, isOutput=True)

    with tile.TileContext(nc) as tc, ExitStack() as ctx:
        const = ctx.enter_context(tc.tile_pool(name="const", bufs=1))
        xTs = [const.tile([128, S], bf16, name=f"xT{k}", tag=f"xT{k}")
               for k in range(KT)]
        eb = const.tile([128, NQ, ST, CH], bf16)  # exp(0.5*binding).T
        wqs = [const.tile([128, DHC], bf16, name=f"wq{k}", tag=f"wq{k}")
               for k in range(KT)]
        wks = [const.tile([128, DHC], bf16, name=f"wk{k}", tag=f"wk{k}")
               for k in range(KT)]
        wvs = [const.tile([128, DHC], bf16, name=f"wv{k}", tag=f"wv{k}")
               for k in range(KT)]
        junk = const.tile([128, CH], bf16)
        wo = const.tile([128, 2, D], bf16)
        bq = const.tile([128, 2], f32)
        bk = const.tile([128, 2], f32)
        qT = const.tile([128, 2, S], bf16)  # [dh, s] head-major
        kT = const.tile([128, 2, S], bf16)
        va = const.tile([128, ST, HPC, 65], bf16)  # v tiles + ones col

        nc.vector.memset(junk[:], 0.0)
        for k in range(KT):
            nc.sync.dma_start(xTs[k][:], xT_e[k * 128:(k + 1) * 128, :])
            nc.sync.dma_start(wks[k][:], wk_e[k * 128:(k + 1) * 128, :])
        for k in range(KT):
            nc.sync.dma_start(wqs[k][:], wq_e[k * 128:(k + 1) * 128, :])
        for k in range(KT):
            nc.sync.dma_start(wvs[k][:], wv_e[k * 128:(k + 1) * 128, :])
        for m in range(2):
            nc.sync.dma_start(bq[:, m:m + 1], bq_e[m * 128:(m + 1) * 128, :])
            nc.sync.dma_start(bk[:, m:m + 1], bk_e[m * 128:(m + 1) * 128, :])
            nc.sync.dma_start(wo[:, m, :], wo_e[m * 128:(m + 1) * 128, :])
        for t in range(ST):
            for n in range(NQ):
                nc.sync.dma_start(
                    eb[:, n, t, :],
                    eb_e[t * 128:(t + 1) * 128, n * CH:(n + 1) * CH])

        T2 = 2  # sk tiles merged per exp/mul instruction
        GS = list(range(0, ST, T2)) + [ST]
        NG = len(GS) - 1
        psS = ctx.enter_context(tc.tile_pool(name="psS", bufs=2, space="PSUM"))
        psX = ctx.enter_context(tc.tile_pool(name="psX", bufs=2, space="PSUM"))
        psA = ctx.enter_context(tc.tile_pool(name="psA", bufs=2, space="PSUM"))
        pP = ctx.enter_context(tc.tile_pool(name="pP", bufs=3))
        pP2 = ctx.enter_context(tc.tile_pool(name="pP2", bufs=5))
        pA = ctx.enter_context(tc.tile_pool(name="pA", bufs=2))
        pR = ctx.enter_context(tc.tile_pool(name="pR", bufs=2))
        pRB = ctx.enter_context(tc.tile_pool(name="pRB", bufs=2))
        pO = ctx.enter_context(tc.tile_pool(name="pO", bufs=3))

        # dummy matmuls warm the PE clock while input DMAs land
        pw = psX.tile([128, CH], f32, tag="px", name="pw")
        for _ in range(10):
            nc.tensor.matmul(pw[:], junk[:, 0:128], junk[:],
                             start=True, stop=True)

        def qk_proj_chunk(which, n):
            w_t, out_t, b_t = (wqs, qT, bq) if which == "q" else (wks, kT, bk)
            for m in range(2):
                pp = psS.tile([128, CH], f32, tag="sc", name="pp")
                for k in range(KT):
                    nc.tensor.matmul(
                        pp[:], w_t[k][:, m * 128:(m + 1) * 128],
                        xTs[k][:, n * CH:(n + 1) * CH],
                        start=(k == 0), stop=(k == KT - 1))
                nc.vector.tensor_scalar_add(
                    out_t[:, m, n * CH:(n + 1) * CH], pp[:], b_t[:, m:m + 1])

        def v_proj_tile(s):
            pv = psS.tile([128, DHC], f32, tag="sc", name="pv")
            for k in range(KT):
                nc.tensor.matmul(
                    pv[:], xTs[k][:, s * 128:(s + 1) * 128], wvs[k][:],
                    start=(k == 0), stop=(k == KT - 1))
            for h in range(HPC):
                nc.vector.tensor_copy(
                    va[:, s, h, 0:64], pv[:, h * 64:(h + 1) * 64])
                nc.gpsimd.memset(va[:, s, h, 64:65], 1.0)

        # upfront: just enough for attention chunk nq=0 to start
        qk_proj_chunk("k", 0)
        qk_proj_chunk("k", 1)
        qk_proj_chunk("q", 0)

        # deferred projection work, drained as PE filler inside the
        # attention loop (keeps TensorE busy while ScalarE runs exp)
        fillers = [lambda s=s: v_proj_tile(s) for s in range(6)]
        fillers[6:6] = []
        fillers.insert(6, lambda: qk_proj_chunk("k", 2))
        fillers.insert(7, lambda: qk_proj_chunk("k", 3))
        fillers.extend([lambda s=s: v_proj_tile(s) for s in range(6, 16)])
        fillers.append(lambda: qk_proj_chunk("q", 1))
        fidx = [0]

        def drain_filler(k=1):
            for _ in range(k):
                if fidx[0] < len(fillers):
                    fillers[fidx[0]]()
                    fidx[0] += 1

        def oproj(nq, att):
            for s4 in range(4):
                for dd in range(2):
                    po = psX.tile([128, CH], f32, tag="px", name="po")
                    for pr in range(2):
                        nc.tensor.matmul(
                            po[:], att[:, pr, s4 * 128:(s4 + 1) * 128],
                            wo[:, pr, dd * CH:(dd + 1) * CH],
                            start=(pr == 0), stop=(pr == 1))
                    ob = pO.tile([128, CH], bf16)
                    nc.scalar.activation(ob[:], po[:], AF.Copy)
                    nc.sync.dma_start(
                        out_e[nq * CH + s4 * 128: nq * CH + (s4 + 1) * 128,
                              dd * CH:(dd + 1) * CH],
                        ob[:])

        prev = None
        for nq in range(NQ):
            if nq == 1:
                fillers.append(lambda: qk_proj_chunk("q", 2))
            elif nq == 2:
                fillers.append(lambda: qk_proj_chunk("q", 3))
            fidx[0] = min(fidx[0], len(fillers))
            att = pA.tile([128, 2, CH], bf16)
            for hp in range(2):
                accs = [psA.tile([65, CH], f32, tag="acc",
                                 name=f"acc{j}")
                        for j in range(2)]
                sco = {}
                p2s = {}
                # software-pipelined over sk tile-groups: scores(g),
                # exp/mul(g-1), attn@v(g-2) — the lag hides the
                # ScalarE->VectorE latency from TensorE
                for g in range(NG + 3):
                    if hp == 0 and g < NG:
                        drain_filler(3 if nq == 0 else 2)
                    if g < NG:
                        sz = GS[g + 1] - GS[g]
                        new = []
                        for j in range(2):
                            ps = psS.tile([128, T2, CH], f32,
                                          tag="sc", name="sc")
                            for u in range(sz):
                                t = GS[g] + u
                                nc.tensor.matmul(
                                    ps[:, u, :],
                                    kT[j * 64:(j + 1) * 64, hp,
                                       t * 128:(t + 1) * 128],
                                    qT[j * 64:(j + 1) * 64, hp,
                                       nq * CH:(nq + 1) * CH],
                                    start=True, stop=True)
                            new.append(ps)
                        sco[g] = new
                    if 1 <= g <= NG:
                        gp = g - 1
                        sz = GS[gp + 1] - GS[gp]
                        cur = []
                        for j in range(2):
                            p = pP.tile([128, T2, CH], bf16)
                            nc.scalar.activation(
                                p[:, :sz, :], sco[gp][j][:, :sz, :],
                                AF.Exp, scale=SCALE)
                            p2 = pP2.tile([128, T2, CH], bf16)
                            nc.vector.tensor_mul(
                                p2[:, :sz, :], p[:, :sz, :],
                                eb[:, nq, GS[gp]:GS[gp] + sz, :])
                            cur.append(p2)
                        p2s[gp] = cur
                        del sco[gp]
                    if g >= 3:
                        ga = g - 3
                        sz = GS[ga + 1] - GS[ga]
                        for j in range(2):
                            h = hp * 2 + j
                            for u in range(sz):
                                t = GS[ga] + u
                                nc.tensor.matmul(
                                    accs[j][:], va[:, t, h, :],
                                    p2s[ga][j][:, u, :],
                                    start=(t == 0), stop=(t == ST - 1))
                        del p2s[ga]
                for j in range(2):
                    rs = pR.tile([1, CH], f32, tag="rs", name="rs")
                    nc.vector.tensor_copy(rs[:], accs[j][64:65, :])
                    r = pR.tile([1, CH], f32)
                    nc.vector.reciprocal_approx_fast(r[:], rs[:])
                    rb = pRB.tile([128, CH], f32)
                    nc.gpsimd.partition_broadcast(rb[:], r[:])
                    nc.vector.tensor_mul(
                        att[j * 64:(j + 1) * 64, hp, :],
                        accs[j][0:64, :], rb[0:64, :])
                if hp == 0 and prev is not None:
                    # previous chunk's o-projection fills the PE during
                    # this chunk's head-pair boundary
                    oproj(prev[0], prev[1])
            prev = (nq, att)
        oproj(prev[0], prev[1])
    nc.compile()
    return nc


def _get_graph():
    if "nc" not in _graph_cache:
        _graph_cache["nc"] = _build()
    return _graph_cache["nc"]


def _prepare_in_maps(inputs):
    x = np.asarray(inputs["x"], np.float32)
    bm = np.asarray(inputs["binding_matrix"], np.float32)
    Wq = np.asarray(inputs["Wq"], np.float32)
    Wk = np.asarray(inputs["Wk"], np.float32)
    Wv = np.asarray(inputs["Wv"], np.float32)
    Wo = np.asarray(inputs["Wo"], np.float32)
    bq = np.asarray(inputs["bq"], np.float32)
    bk = np.asarray(inputs["bk"], np.float32)

    expbT = np.exp(0.5 * bm.T).astype(BF16)
    xTs = [np.ascontiguousarray(x[b].T).astype(BF16) for b in range(B)]
    in_maps = []
    for c in range(NCORES):
        b, g = divmod(c, 4)
        sl = slice(g * DHC, (g + 1) * DHC)
        in_maps.append({
            "xT": xTs[b],
            "wq": np.ascontiguousarray(Wq[:, sl]).astype(BF16),
            "wk": np.ascontiguousarray(Wk[:, sl]).astype(BF16),
            "wv": np.ascontiguousarray(Wv[:, sl]).astype(BF16),
            "wo": np.ascontiguousarray(Wo[sl, :]).astype(BF16),
            "expbT": expbT,
            "bq": np.ascontiguousarray(bq[sl]).reshape(DHC, 1),
            "bk": np.ascontiguousarray(bk[sl]).reshape(DHC, 1),
        })
    return in_maps


def _install_trace_hooks():
    """The container image's antenv stub lacks axon_hooks; synthesize it so
    run_bass_kernel_spmd(trace=True) can reach the NTFF profiler in
    libaxon_pjrt.so, and neuter the bucket artifact upload."""
    import types

    try:
        from antenv.axon_hooks import get_axon_ntff_profile_hook  # noqa: F401
    except ImportError:
        import antenv

        m = types.ModuleType("antenv.axon_hooks")
        m._hook = None
        m.set_axon_ntff_profile_hook = lambda h: setattr(m, "_hook", h)
        m.get_axon_ntff_profile_hook = lambda: m._hook
        sys.modules["antenv.axon_hooks"] = m
        antenv.axon_hooks = m
        if "/root/.axon_site" not in sys.path:
            sys.path.insert(0, "/root/.axon_site")
        from trn_agent_boot.trn_boot import _ntff_profile_via_ctypes

        m._hook = _ntff_profile_via_ctypes("/opt/axon/libaxon_pjrt.so")
    import concourse.bass_utils as bu

    bu.upload_artifacts = lambda tmpdir: str(tmpdir)


def run(inputs, trace=False, tmpdir=None):
    from concourse.bass_utils import run_bass_kernel_spmd

    if trace:
        _install_trace_hooks()
    nc = _get_graph()
    in_maps = _prepare_in_maps(inputs)
    res = run_bass_kernel_spmd(nc, in_maps, list(range(NCORES)), trace=trace,
                               tmpdir=tmpdir)

    bv = np.asarray(inputs["bv"], np.float32)
    bo = np.asarray(inputs["bo"], np.float32)
    Wo = np.asarray(inputs["Wo"], np.float32)
    const_vec = (bv @ Wo + bo).astype(np.float32)

    out = np.empty((B, S, D), np.float32)
    for b in range(B):
        acc = np.zeros((S, D), np.float32)
        for g in range(4):
            acc += np.asarray(res.results[b * 4 + g]["out"], np.float32)
        out[b] = acc + const_vec
    return out, res


def kernel(**inputs):
    out, _ = run(inputs, trace=False)
    return out



# revision 5
# speedup vs baseline: 1.0852x; 1.0852x over previous
"""AttentionWithBinding distributed Bass kernel for 8 TRN2 NeuronCores.

Sharding: 8 cores = 2 batches x 4 head-groups (4 heads / 256 dims each).
Per core: q/k/v projections (weight-stationary matmuls from a host
pre-transposed xT), flash-style attention in scoresT [sk, sq] orientation,
softmax exp on ScalarE with the additive binding bias folded in as a
host-precomputed exp(0.5*binding.T) bf16 multiplier on VectorE, row-sums
fused into the attn@v matmul via a ones-column on v, and the per-head
o-projection partials. Host sums the 4 partials per batch and adds the
analytic bias vector bv@Wo + bo.

Inner-loop structure (per 512-wide q chunk, per head-pair): one period per
sk tile t. The two heads of the pair (j=0/1) share one [128, 2, 512] PSUM
score tile, so their QK^T matmuls land in different PE row-groups
(tile_position (0,0)/(64,0)) and run concurrently, one ScalarE ACTIVATE
exps both, and one broadcast VectorE multiply applies the binding term.
ScalarE does nothing but exp (the pacing op); PSUM->SBUF copies live on
GpSimdE/VectorE. DMA issue order tracks consumption order so the pipeline
starts as early as the HBM stream allows.
"""

import sys

sys.path.insert(0, "/opt/trn_rl_repo")

import numpy as np
import ml_dtypes
from contextlib import ExitStack

BF16 = ml_dtypes.bfloat16

B, S, D = 2, 2048, 1024
H, HD = 16, 64
HPC = 4  # heads per core
DHC = HPC * HD  # 256 head dims per core
SCALE = HD ** -0.5
NCORES = 8
KT = D // 128  # 8 contraction tiles over D
ST = S // 128  # 16 tiles over S
CH = 512  # free-dim chunk (one PSUM bank of f32)
NQ = S // CH  # 4 query chunks

_graph_cache = {}


def _build():
    import concourse.bacc as bacc
    import concourse.mybir as mybir
    from concourse import tile
    from concourse.bass import broadcast_tensor_aps

    f32 = mybir.dt.float32
    bf16 = mybir.dt.bfloat16
    AF = mybir.ActivationFunctionType

    nc = bacc.Bacc(None)

    xT_e = nc.declare_dram_parameter("xT", [D, S], bf16, isOutput=False)
    wq_e = nc.declare_dram_parameter("wq", [D, DHC], bf16, isOutput=False)
    wk_e = nc.declare_dram_parameter("wk", [D, DHC], bf16, isOutput=False)
    wv_e = nc.declare_dram_parameter("wv", [D, DHC], bf16, isOutput=False)
    wo_e = nc.declare_dram_parameter("wo", [DHC, D], bf16, isOutput=False)
    eb_e = nc.declare_dram_parameter("expbT", [S, S], bf16, isOutput=False)
    bq_e = nc.declare_dram_parameter("bq", [DHC, 1], f32, isOutput=False)
    bk_e = nc.declare_dram_parameter("bk", [DHC, 1], f32, isOutput=False)
    out_e = nc.declare_dram_parameter("out", [S, D], bf16, isOutput=True)

    with tile.TileContext(nc) as tc, ExitStack() as ctx:
        const = ctx.enter_context(tc.tile_pool(name="const", bufs=1))
        xTs = [const.tile([128, S], bf16, name=f"xT{k}", tag=f"xT{k}")
               for k in range(KT)]
        eb = const.tile([128, NQ, ST, CH], bf16)  # exp(0.5*binding).T
        wqs = [const.tile([128, DHC], bf16, name=f"wq{k}", tag=f"wq{k}")
               for k in range(KT)]
        wks = [const.tile([128, DHC], bf16, name=f"wk{k}", tag=f"wk{k}")
               for k in range(KT)]
        wvs = [const.tile([128, DHC], bf16, name=f"wv{k}", tag=f"wv{k}")
               for k in range(KT)]
        junk = const.tile([128, CH], bf16)
        wo = const.tile([128, 2, D], bf16)
        bq = const.tile([128, 2], f32)
        bk = const.tile([128, 2], f32)
        qT = const.tile([128, 2, S], bf16)  # [dh, hp, s] head-pair-major
        kT = const.tile([128, 2, S], bf16)
        va = const.tile([128, ST, HPC, 65], bf16)  # v tiles + ones col

        nc.vector.memset(junk[:], 0.0)
        # DMA issue order == consumption order: x/wk first (k-proj feeds the
        # score pipeline), wv (v fillers run early in the loop), wq (q chunk
        # 0 gates loop start), binding tiles for q-chunk 0, wo, rest of eb.
        for k in range(KT):
            nc.sync.dma_start(xTs[k][:], xT_e[k * 128:(k + 1) * 128, :])
            nc.sync.dma_start(wks[k][:], wk_e[k * 128:(k + 1) * 128, :])
        for k in range(KT):
            nc.sync.dma_start(wvs[k][:], wv_e[k * 128:(k + 1) * 128, :])
        for k in range(KT):
            nc.sync.dma_start(wqs[k][:], wq_e[k * 128:(k + 1) * 128, :])
        for m in range(2):
            nc.sync.dma_start(bq[:, m:m + 1], bq_e[m * 128:(m + 1) * 128, :])
            nc.sync.dma_start(bk[:, m:m + 1], bk_e[m * 128:(m + 1) * 128, :])
        for t in range(ST):
            nc.sync.dma_start(
                eb[:, 0, t, :], eb_e[t * 128:(t + 1) * 128, 0:CH])
        for m in range(2):
            nc.sync.dma_start(wo[:, m, :], wo_e[m * 128:(m + 1) * 128, :])
        for n in range(1, NQ):
            for t in range(ST):
                nc.sync.dma_start(
                    eb[:, n, t, :],
                    eb_e[t * 128:(t + 1) * 128, n * CH:(n + 1) * CH])

        psS = ctx.enter_context(tc.tile_pool(name="psS", bufs=2, space="PSUM"))
        psX = ctx.enter_context(tc.tile_pool(name="psX", bufs=2, space="PSUM"))
        psA = ctx.enter_context(tc.tile_pool(name="psA", bufs=2, space="PSUM"))
        pP = ctx.enter_context(tc.tile_pool(name="pP", bufs=3))
        pP2 = ctx.enter_context(tc.tile_pool(name="pP2", bufs=4))
        pA = ctx.enter_context(tc.tile_pool(name="pA", bufs=2))
        pR = ctx.enter_context(tc.tile_pool(name="pR", bufs=2))
        pRB = ctx.enter_context(tc.tile_pool(name="pRB", bufs=2))
        pO = ctx.enter_context(tc.tile_pool(name="pO", bufs=4))

        # dummy matmuls keep the PE HAM-warm while the input DMAs land
        for _ in range(28):
            pw = psX.tile([128, CH], f32, tag="px", name="pw")
            nc.tensor.matmul(pw[:], junk[:, 0:128], junk[:],
                             start=True, stop=True)

        def qk_proj_unit(which, n, m):
            # one 128-col half (m) of one 512-wide q/k projection chunk (n)
            w_t, out_t, b_t = (wqs, qT, bq) if which == "q" else (wks, kT, bk)
            pp = psX.tile([128, CH], f32, tag="px", name="pp")
            for k in range(KT):
                nc.tensor.matmul(
                    pp[:], w_t[k][:, m * 128:(m + 1) * 128],
                    xTs[k][:, n * CH:(n + 1) * CH],
                    start=(k == 0), stop=(k == KT - 1))
            nc.vector.tensor_scalar_add(
                out_t[:, m, n * CH:(n + 1) * CH], pp[:], b_t[:, m:m + 1])

        def v_proj_tile(s):
            pv = psX.tile([128, HPC, 64], f32, tag="px", name="pv")
            for k in range(KT):
                nc.tensor.matmul(
                    pv[:], xTs[k][:, s * 128:(s + 1) * 128], wvs[k][:],
                    start=(k == 0), stop=(k == KT - 1))
            nc.vector.tensor_copy(va[:, s, :, 0:64], pv[:])
            nc.gpsimd.memset(va[:, s, :, 64:65], 1.0)

        # upfront: just enough for attention (nq=0, hp=0) to start
        qk_proj_unit("k", 0, 0)
        qk_proj_unit("k", 1, 0)
        qk_proj_unit("q", 0, 0)

        # deferred projection work drained as PE filler inside the attention
        # loop, ordered by first-need period (v_t at period t+1; k chunk c
        # m=0 at period 4c; all m=1 halves by period 16; q chunk c by 32c)
        fillers = []
        fillers += [lambda s=s: v_proj_tile(s) for s in (0, 1)]
        fillers.append(lambda: qk_proj_unit("k", 2, 0))
        fillers += [lambda s=s: v_proj_tile(s) for s in (2, 3)]
        fillers.append(lambda: qk_proj_unit("k", 3, 0))
        fillers += [lambda s=s: v_proj_tile(s) for s in (4, 5)]
        fillers.append(lambda: qk_proj_unit("q", 0, 1))
        fillers += [lambda s=s: v_proj_tile(s) for s in (6, 7)]
        fillers.append(lambda: qk_proj_unit("k", 0, 1))
        fillers += [lambda s=s: v_proj_tile(s) for s in (8, 9)]
        fillers.append(lambda: qk_proj_unit("k", 1, 1))
        fillers += [lambda s=s: v_proj_tile(s) for s in (10, 11)]
        fillers.append(lambda: qk_proj_unit("k", 2, 1))
        fillers += [lambda s=s: v_proj_tile(s) for s in (12, 13)]
        fillers.append(lambda: qk_proj_unit("k", 3, 1))
        fillers += [lambda s=s: v_proj_tile(s) for s in (14, 15)]
        for n in range(1, NQ):
            fillers.append(lambda n=n: qk_proj_unit("q", n, 0))
            fillers.append(lambda n=n: qk_proj_unit("q", n, 1))
        fidx = [0]

        def drain_filler(k=1):
            for _ in range(k):
                if fidx[0] < len(fillers):
                    fillers[fidx[0]]()
                    fidx[0] += 1

        def oproj_piece(nq, att, piece):
            s4, dd = divmod(piece, 2)
            po = psX.tile([128, CH], f32, tag="px", name="po")
            for pr in range(2):
                nc.tensor.matmul(
                    po[:], att[:, pr, s4 * 128:(s4 + 1) * 128],
                    wo[:, pr, dd * CH:(dd + 1) * CH],
                    start=(pr == 0), stop=(pr == 1))
            ob = pO.tile([128, CH], bf16)
            nc.vector.tensor_copy(ob[:], po[:])
            nc.sync.dma_start(
                out_e[nq * CH + s4 * 128: nq * CH + (s4 + 1) * 128,
                      dd * CH:(dd + 1) * CH],
                ob[:])

        prev = None
        for nq in range(NQ):
            att = pA.tile([128, 2, CH], bf16)
            for hp in range(2):
                accs = [psA.tile([65, CH], f32, tag="acc", name=f"acc{j}")
                        for j in range(2)]
                sco = {}
                p2s = {}
                # software-pipelined over sk tiles: scores(t), exp/mul(t-1),
                # attn@v(t-3); the lag hides ScalarE->VectorE latency
                for t in range(ST + 3):
                    if t < ST:
                        ps = psS.tile([128, 2, CH], f32, tag="sc", name="sc")
                        for j in range(2):
                            nc.tensor.matmul(
                                ps[:, j, :],
                                kT[j * 64:(j + 1) * 64, hp,
                                   t * 128:(t + 1) * 128],
                                qT[j * 64:(j + 1) * 64, hp,
                                   nq * CH:(nq + 1) * CH],
                                start=True, stop=True)
                        sco[t] = ps
                    if 1 <= t <= ST:
                        tp = t - 1
                        p = pP.tile([128, 2, CH], bf16)
                        nc.scalar.activation(p[:], sco[tp][:], AF.Exp,
                                             scale=SCALE)
                        p2 = pP2.tile([128, 2, CH], bf16)
                        for j in range(2):
                            nc.vector.tensor_mul(p2[:, j, :], p[:, j, :],
                                                 eb[:, nq, tp, :])
                        p2s[tp] = p2
                        del sco[tp]
                    if t >= 3:
                        ta = t - 3
                        for j in range(2):
                            h = hp * 2 + j
                            nc.tensor.matmul(
                                accs[j][:], va[:, ta, h, :], p2s[ta][:, j, :],
                                start=(ta == 0), stop=(ta == ST - 1))
                        del p2s[ta]
                    if t < ST:
                        drain_filler(2 if (nq == 0 and hp == 0 and t < 12)
                                     else 1)
                        if hp == 0 and prev is not None and t % 2 == 0:
                            oproj_piece(prev[0], prev[1], t // 2)
                for j in range(2):
                    rs = pR.tile([1, CH], f32, tag="rs", name="rs")
                    nc.vector.tensor_copy(rs[:], accs[j][64:65, :])
                    r = pR.tile([1, CH], f32)
                    nc.vector.reciprocal_approx_fast(r[:], rs[:])
                    rb = pRB.tile([128, CH], f32)
                    nc.gpsimd.partition_broadcast(rb[:], r[:])
                    nc.vector.tensor_mul(
                        att[j * 64:(j + 1) * 64, hp, :],
                        accs[j][0:64, :], rb[0:64, :])
            prev = (nq, att)
        for piece in range(8):
            oproj_piece(prev[0], prev[1], piece)
    nc.compile()
    return nc


def _get_graph():
    if "nc" not in _graph_cache:
        _graph_cache["nc"] = _build()
    return _graph_cache["nc"]


def _prepare_in_maps(inputs):
    x = np.asarray(inputs["x"], np.float32)
    bm = np.asarray(inputs["binding_matrix"], np.float32)
    Wq = np.asarray(inputs["Wq"], np.float32)
    Wk = np.asarray(inputs["Wk"], np.float32)
    Wv = np.asarray(inputs["Wv"], np.float32)
    Wo = np.asarray(inputs["Wo"], np.float32)
    bq = np.asarray(inputs["bq"], np.float32)
    bk = np.asarray(inputs["bk"], np.float32)

    expbT = np.exp(0.5 * bm.T).astype(BF16)
    xTs = [np.ascontiguousarray(x[b].T).astype(BF16) for b in range(B)]
    in_maps = []
    for c in range(NCORES):
        b, g = divmod(c, 4)
        sl = slice(g * DHC, (g + 1) * DHC)
        in_maps.append({
            "xT": xTs[b],
            "wq": np.ascontiguousarray(Wq[:, sl]).astype(BF16),
            "wk": np.ascontiguousarray(Wk[:, sl]).astype(BF16),
            "wv": np.ascontiguousarray(Wv[:, sl]).astype(BF16),
            "wo": np.ascontiguousarray(Wo[sl, :]).astype(BF16),
            "expbT": expbT,
            "bq": np.ascontiguousarray(bq[sl]).reshape(DHC, 1),
            "bk": np.ascontiguousarray(bk[sl]).reshape(DHC, 1),
        })
    return in_maps


def _install_trace_hooks():
    """The container image's antenv stub lacks axon_hooks; synthesize it so
    run_bass_kernel_spmd(trace=True) can reach the NTFF profiler in
    libaxon_pjrt.so, and neuter the bucket artifact upload."""
    import types

    try:
        from antenv.axon_hooks import get_axon_ntff_profile_hook  # noqa: F401
    except ImportError:
        import antenv

        m = types.ModuleType("antenv.axon_hooks")
        m._hook = None
        m.set_axon_ntff_profile_hook = lambda h: setattr(m, "_hook", h)
        m.get_axon_ntff_profile_hook = lambda: m._hook
        sys.modules["antenv.axon_hooks"] = m
        antenv.axon_hooks = m
        if "/root/.axon_site" not in sys.path:
            sys.path.insert(0, "/root/.axon_site")
        from trn_agent_boot.trn_boot import _ntff_profile_via_ctypes

        m._hook = _ntff_profile_via_ctypes("/opt/axon/libaxon_pjrt.so")
    import concourse.bass_utils as bu

    bu.upload_artifacts = lambda tmpdir: str(tmpdir)


def run(inputs, trace=False, tmpdir=None):
    from concourse.bass_utils import run_bass_kernel_spmd

    if trace:
        _install_trace_hooks()
    nc = _get_graph()
    in_maps = _prepare_in_maps(inputs)
    res = run_bass_kernel_spmd(nc, in_maps, list(range(NCORES)), trace=trace,
                               tmpdir=tmpdir)

    bv = np.asarray(inputs["bv"], np.float32)
    bo = np.asarray(inputs["bo"], np.float32)
    Wo = np.asarray(inputs["Wo"], np.float32)
    const_vec = (bv @ Wo + bo).astype(np.float32)

    out = np.empty((B, S, D), np.float32)
    for b in range(B):
        acc = np.zeros((S, D), np.float32)
        for g in range(4):
            acc += np.asarray(res.results[b * 4 + g]["out"], np.float32)
        out[b] = acc + const_vec
    return out, res


def kernel(**inputs):
    out, _ = run(inputs, trace=False)
    return out
